# revision 28
# baseline (speedup 1.0000x reference)
"""Self-contained Trainium2 kernel for nn_BRA_32220844655457 (sparse/regional
attention).

Reference computation (B=4, N=4000, C=D=1024, 5 regions of 800 keys):
    Q = x @ Wq.T + bq ; K = x @ Wk.T + bk ; V = x @ Wv.T + bv
    S = Q @ K.T                      (per batch, (4000, 4000))
    P = softmax(S per (query, 800-key region))
    out = (sum_regions P_g @ V_g) @ Wo.T + bo

Sharding: 8 cores = 4 batches x 2 query-halves (2000 queries per core).
Each core recomputes K/V for its batch (no cross-core communication).

Per-core pipeline:
  phase 1: project Q^T first (so its DRAM spill/reload overlaps the rest),
           then K^T and V (bf16) in one fused pass over 512-column x chunks
           (V stationary slices at 128-element offsets -- fp32r stationary
           operands require 32-byte-aligned offsets on HW); spill to DRAM.
  phase 2: stream regions (g outer, q-tile inner): scores (fp32r),
           softmax on free axis, PE-transpose P (bf16), P@V accumulated in
           PSUM per region, region results summed in SBUF (bf16).
  phase 3: PE-transpose accumulator, project with Wo (bf16), write out.

Precision: the softmax logit chain (x, Wq, Wk, Q^T, K^T, scores) runs in
float32r (TF32-like, ~1e-4 rel) because logits have std ~32 with no 1/sqrt(d)
scaling -- bf16 logits would randomly reorder near-ties in the per-region
softmax. The V/output side is linear in the inputs, so bf16 there only
contributes ~0.3% relative error.
"""

import numpy as np
from contextlib import ExitStack

import concourse.bacc as bacc
import concourse.tile as tile
import concourse.mybir as mybir
from concourse import bass_utils
from concourse.masks import make_identity

f32 = mybir.dt.float32
f32r = mybir.dt.float32r
bf16 = mybir.dt.bfloat16

B, N, C, D = 4, 4000, 1024, 1024
G, RS = 5, 800          # regions, region size
NCORES = 8
NQ = N // 2             # queries per core
CC = C // 128           # c chunks
DC = D // 128           # d chunks
JB = 500                # spill column chunk for Q^T
Q_STARTS = [min(i * 128, NQ - 128) for i in range((NQ + 127) // 128)]  # 16 tiles
# region j-chunks: starts/widths within a region (RS=800 -> 6x128 + 32)
RJ = []
_j = 0
while _j < RS:
    w = min(128, RS - _j)
    RJ.append((_j, w))
    _j += w

_NC_CACHE = {}


def _build_nc():
    if "nc" in _NC_CACHE:
        return _NC_CACHE["nc"]
    nc = bacc.Bacc("TRN2", target_bir_lowering=False, debug=False,
                   num_devices=NCORES)

    xT = nc.dram_tensor("xT", [C, N], f32r, kind="ExternalInput").ap()
    xqT = nc.dram_tensor("xqT", [C, NQ], f32r, kind="ExternalInput").ap()
    wqT = nc.dram_tensor("wqT", [C, D], f32r, kind="ExternalInput").ap()
    wkT = nc.dram_tensor("wkT", [C, D], f32r, kind="ExternalInput").ap()
    wvT = nc.dram_tensor("wvT", [C, D], f32r, kind="ExternalInput").ap()
    woT = nc.dram_tensor("woT", [D, D], bf16, kind="ExternalInput").ap()
    bq = nc.dram_tensor("bq", [1, D], f32r, kind="ExternalInput").ap()
    bk = nc.dram_tensor("bk", [1, D], f32r, kind="ExternalInput").ap()
    bv = nc.dram_tensor("bv", [1, D], f32r, kind="ExternalInput").ap()
    bo = nc.dram_tensor("bo", [1, D], bf16, kind="ExternalInput").ap()
    out = nc.dram_tensor("out", [NQ, D], f32, kind="ExternalOutput").ap()

    with tile.TileContext(nc) as tc, ExitStack() as ctx:
        # ---- pools that live for the whole kernel ----
        const = ctx.enter_context(tc.tile_pool(name="const", bufs=1))
        stats = ctx.enter_context(tc.tile_pool(name="stats", bufs=8))
        ps_s = ctx.enter_context(tc.tile_pool(name="ps_s", bufs=2, space="PSUM"))
        ps_acc = ctx.enter_context(tc.tile_pool(name="ps_acc", bufs=1, space="PSUM"))
        ps_pt = ctx.enter_context(tc.tile_pool(name="ps_pt", bufs=1, space="PSUM"))
        dram = ctx.enter_context(tc.tile_pool(name="dram", bufs=1, space="DRAM"))

        kt_sp = dram.tile([C, N], f32r, tag="kt_sp")
        v_sp = dram.tile([N, D], bf16, tag="v_sp")
        qt_sp = dram.tile([C, NQ], f32r, tag="qt_sp")

        ident = const.tile([128, 128], bf16, tag="ident")
        make_identity(nc, ident[:])
        ones_b = const.tile([1, 128], bf16, tag="ones_b")
        nc.vector.memset(ones_b[:], 1.0)
        bo_sb = const.tile([1, D], bf16, tag="bo")
        nc.sync.dma_start(bo_sb[:], bo)

        # ================= phase 1: projections =================
        with tc.tile_pool(name="p1const", bufs=1) as p1c, \
             tc.tile_pool(name="wpool", bufs=64) as wp, \
             tc.tile_pool(name="xkpool", bufs=16) as xkp, \
             tc.tile_pool(name="stg_r_pool", bufs=4) as stgr, \
             tc.tile_pool(name="stg_b_pool", bufs=3) as stgb:

            ones_f = p1c.tile([1, 512], f32, tag="ones_f")
            nc.vector.memset(ones_f[:], 1.0)
            ones_r = p1c.tile([1, 512], f32r, tag="ones_r")
            nc.vector.tensor_copy(ones_r[:], ones_f[:])
            bq_sb = p1c.tile([1, D], f32r, tag="bq")
            nc.sync.dma_start(bq_sb[:], bq)
            bk_sb = p1c.tile([1, D], f32r, tag="bk")
            nc.sync.dma_start(bk_sb[:], bk)
            bv_sb = p1c.tile([1, D], f32r, tag="bv")
            nc.sync.dma_start(bv_sb[:], bv)

            # ---- Q^T = (wqT.T @ xqT) + bq, spilled as (D rows, NQ cols) ----
            # reuse the weight pool slots (same tag) for wq
            wq_t = [[None] * CC for _ in range(DC)]
            for dc in range(DC):
                for cc in range(CC):
                    t = wp.tile([128, 128], f32r, tag="w",
                                name=f"wq{dc}_{cc}")
                    nc.sync.dma_start(
                        t[:], wqT[cc * 128:(cc + 1) * 128,
                                  dc * 128:(dc + 1) * 128])
                    wq_t[dc][cc] = t
            for qc in range(NQ // JB):
                xq_t = []
                for cc in range(CC):
                    t = xkp.tile([128, JB], f32r, tag="xk", name=f"xq{cc}")
                    nc.sync.dma_start(
                        t[:], xqT[cc * 128:(cc + 1) * 128,
                                  qc * JB:(qc + 1) * JB])
                    xq_t.append(t)
                for dc in range(DC):
                    ps = ps_s.tile([128, 1024], f32, tag="s", name="psq")
                    for cc in range(CC):
                        nc.tensor.matmul(
                            ps[:, 0:JB], wq_t[dc][cc][:], xq_t[cc][:],
                            start=(cc == 0), stop=False)
                    nc.tensor.matmul(
                        ps[:, 0:JB], bq_sb[0:1, dc * 128:(dc + 1) * 128],
                        ones_r[:, 0:JB], start=False, stop=True)
                    st = stgr.tile([128, JB], f32r, tag="stg_r", name="stq")
                    nc.scalar.copy(st[:], ps[:, 0:JB])
                    nc.sync.dma_start(
                        qt_sp[dc * 128:(dc + 1) * 128, qc * JB:(qc + 1) * JB],
                        st[:])

            # ---- fused K^T + V pass over one x stream ----
            # K^T = (wkT.T @ xT) + bk spilled as (D rows, N cols);
            # V = (xT.T @ wvT) + bv (bf16) from the same x tiles, as
            # 128-row j-subtiles (32-byte-aligned fp32r stationary slices).
            wk_t = [[None] * CC for _ in range(DC)]
            for dc in range(DC):
                for cc in range(CC):
                    t = wp.tile([128, 128], f32r, tag="w",
                                name=f"wk{dc}_{cc}")
                    nc.sync.dma_start(
                        t[:], wkT[cc * 128:(cc + 1) * 128,
                                  dc * 128:(dc + 1) * 128])
                    wk_t[dc][cc] = t
            wv_t = []
            for cc in range(CC):
                t = wp.tile([128, D], f32r, tag="wv", bufs=CC,
                            name=f"wv{cc}")
                nc.sync.dma_start(t[:], wvT[cc * 128:(cc + 1) * 128, :])
                wv_t.append(t)
            # 512-col x chunks so the V stationary slices sit at 128-element
            # (512-byte) offsets -- fp32r weight operands at unaligned
            # offsets produced garbage on HW (see memory notes).
            KCH = []
            _c0 = 0
            while _c0 < N:
                KCH.append((_c0, min(512, N - _c0)))
                _c0 += 512
            for (c0, cw) in KCH:
                xk_t = []
                for cc in range(CC):
                    t = xkp.tile([128, 512], f32r, tag="xk", name=f"xk{cc}")
                    nc.sync.dma_start(
                        t[:, 0:cw], xT[cc * 128:(cc + 1) * 128, c0:c0 + cw])
                    xk_t.append(t)
                for dc in range(DC):
                    ps = ps_s.tile([128, 1024], f32, tag="s", name="psk")
                    for cc in range(CC):
                        nc.tensor.matmul(
                            ps[:, 0:cw], wk_t[dc][cc][:], xk_t[cc][:, 0:cw],
                            start=(cc == 0), stop=False)
                    nc.tensor.matmul(
                        ps[:, 0:cw], bk_sb[0:1, dc * 128:(dc + 1) * 128],
                        ones_r[:, 0:cw], start=False, stop=True)
                    st = stgr.tile([128, 512], f32r, tag="stg_r", name="stk")
                    nc.scalar.copy(st[:, 0:cw], ps[:, 0:cw])
                    nc.sync.dma_start(
                        kt_sp[dc * 128:(dc + 1) * 128, c0:c0 + cw],
                        st[:, 0:cw])
                vo = 0
                while vo < cw:
                    vw = min(128, cw - vo)
                    ps = ps_s.tile([128, 1024], f32, tag="s", name="psv")
                    for nh in range(2):
                        sl = slice(nh * 512, (nh + 1) * 512)
                        for cc in range(CC):
                            nc.tensor.matmul(
                                ps[0:vw, sl],
                                xk_t[cc][:, vo:vo + vw],
                                wv_t[cc][:, sl], start=(cc == 0), stop=False)
                        nc.tensor.matmul(
                            ps[0:vw, sl], ones_r[0:1, 0:vw], bv_sb[0:1, sl],
                            start=False, stop=True)
                    st = stgb.tile([128, 1024], bf16, tag="stg_b", name="stv")
                    nc.scalar.copy(st[0:vw, :], ps[0:vw, :])
                    nc.sync.dma_start(
                        v_sp[c0 + vo:c0 + vo + vw, :], st[0:vw, :])
                    vo += vw

        # ================= phase 2 + 3 =================
        with tc.tile_pool(name="outpool", bufs=len(Q_STARTS)) as op, \
             tc.tile_pool(name="ptpool", bufs=8) as ptp:

            out_sb = [op.tile([128, D], bf16, tag="out", name=f"out{i}")
                      for i in range(len(Q_STARTS))]

            with tc.tile_pool(name="qtpool", bufs=DC) as qtp, \
                 tc.tile_pool(name="ktpool", bufs=16) as ktp, \
                 tc.tile_pool(name="vpool", bufs=14) as vp, \
                 tc.tile_pool(name="ppool", bufs=3) as pp, \
                 tc.tile_pool(name="pbpool", bufs=3) as pbp:

                qt_t = []
                for dc in range(DC):
                    t = qtp.tile([128, NQ], f32r, tag="qt", name=f"qt{dc}")
                    nc.sync.dma_start(t[:], qt_sp[dc * 128:(dc + 1) * 128, :])
                    qt_t.append(t)

                for g in range(G):
                    kt_g = []
                    for dc in range(DC):
                        t = ktp.tile([128, RS], f32r, tag="kt",
                                     name=f"kt{g}_{dc}")
                        nc.sync.dma_start(
                            t[:], kt_sp[dc * 128:(dc + 1) * 128,
                                        g * RS:(g + 1) * RS])
                        kt_g.append(t)
                    v_g = []
                    for vi, (j0, jw) in enumerate(RJ):
                        t = vp.tile([128, D], bf16, tag="v",
                                    name=f"v{g}_{vi}")
                        nc.sync.dma_start(
                            t[0:jw, :],
                            v_sp[g * RS + j0:g * RS + j0 + jw, :])
                        v_g.append(t)

                    for qi, q0 in enumerate(Q_STARTS):
                        # scores (128 q, 800 j) in two banks [0:400],[512:912]
                        s_ps = ps_s.tile([128, 1024], f32, tag="s", name="ss")
                        for h in range(2):
                            o = h * 512
                            ksl = slice(h * 400, (h + 1) * 400)
                            for dc in range(DC):
                                nc.tensor.matmul(
                                    s_ps[:, o:o + 400],
                                    qt_t[dc][:, q0:q0 + 128], kt_g[dc][:, ksl],
                                    start=(dc == 0), stop=(dc == DC - 1))
                        sv = s_ps[:, :].rearrange(
                            "p (b x) -> p b x", b=2)[:, :, 0:400]
                        negm = stats.tile([128, 1], f32, tag="negm",
                                          name="negm")
                        nc.vector.tensor_reduce(
                            negm[:], sv, axis=mybir.AxisListType.XY,
                            op=mybir.AluOpType.max, negate=True)
                        p_f = pp.tile([128, RS], f32, tag="p", name="pf")
                        lsum = stats.tile([128, 1], f32, tag="l", name="lsum")
                        pv = p_f[:, :].rearrange("p (b x) -> p b x", b=2)
                        nc.scalar.activation(
                            pv, sv, mybir.ActivationFunctionType.Exp,
                            bias=negm[:], scale=1.0, accum_out=lsum[:])
                        rsum = stats.tile([128, 1], f32, tag="r", name="rsum")
                        nc.vector.reciprocal(rsum[:], lsum[:])
                        p_b = pbp.tile([128, RS], bf16, tag="pb", name="pb")
                        nc.vector.tensor_scalar_mul(p_b[:], p_f[:], rsum[:])

                        av_ps = ps_acc.tile([128, 1024], f32, tag="acc",
                                            name="av")
                        for ji, (j0, jw) in enumerate(RJ):
                            pt_sb = ptp.tile([128, 128], bf16, tag="pt_sb",
                                             name="pts")
                            if jw == 128:
                                nc.sync.dma_start_transpose(
                                    pt_sb[0:128, 0:128],
                                    p_b[:, j0:j0 + 128])
                            else:
                                pt_ps = ps_pt.tile([128, 128], bf16,
                                                   tag="pt", name="ptp")
                                nc.tensor.transpose(
                                    pt_ps[0:jw, 0:128], p_b[:, j0:j0 + jw],
                                    ident[:])
                                nc.scalar.copy(pt_sb[0:jw, :],
                                               pt_ps[0:jw, 0:128])
                            for nh in range(2):
                                sl = slice(nh * 512, (nh + 1) * 512)
                                nc.tensor.matmul(
                                    av_ps[:, sl], pt_sb[0:jw, :],
                                    v_g[ji][0:jw, sl],
                                    start=(ji == 0), stop=(ji == len(RJ) - 1))
                        if g == 0:
                            nc.vector.tensor_copy(out_sb[qi][:], av_ps[:])
                        else:
                            nc.vector.tensor_tensor(
                                out_sb[qi][:], out_sb[qi][:], av_ps[:],
                                op=mybir.AluOpType.add)

            # ---------------- phase 3: output projection ----------------
            with tc.tile_pool(name="wopool", bufs=DC) as wop, \
                 tc.tile_pool(name="otpool", bufs=10) as otp, \
                 tc.tile_pool(name="stg_f_pool", bufs=3) as stgf:
                wo_t = []
                for dc in range(DC):
                    t = wop.tile([128, D], bf16, tag="wo", name=f"wo{dc}")
                    nc.sync.dma_start(t[:], woT[dc * 128:(dc + 1) * 128, :])
                    wo_t.append(t)

                for qi, q0 in enumerate(Q_STARTS):
                    ot_t = []
                    for dc in range(DC):
                        ot = otp.tile([128, 128], bf16, tag="ot",
                                      name=f"ot{dc}")
                        nc.sync.dma_start_transpose(
                            ot[:, 0:128],
                            out_sb[qi][:, dc * 128:(dc + 1) * 128])
                        ot_t.append(ot)
                    f_ps = ps_acc.tile([128, 1024], f32, tag="acc", name="fps")
                    for nh in range(2):
                        sl = slice(nh * 512, (nh + 1) * 512)
                        for dc in range(DC):
                            nc.tensor.matmul(
                                f_ps[:, sl], ot_t[dc][:], wo_t[dc][:, sl],
                                start=(dc == 0), stop=False)
                        nc.tensor.matmul(
                            f_ps[:, sl], ones_b[:], bo_sb[0:1, sl],
                            start=False, stop=True)
                    st = stgf.tile([128, 1024], f32, tag="stg_f", name="stf")
                    nc.scalar.copy(st[:], f_ps[:])
                    if qi > 0 and q0 < Q_STARTS[qi - 1] + 128:
                        lo = Q_STARTS[qi - 1] + 128 - q0
                        nc.sync.dma_start(out[q0 + lo:q0 + 128, :],
                                          st[lo:128, :])
                    else:
                        nc.sync.dma_start(out[q0:q0 + 128, :], st[:])

    nc.compile()
    _NC_CACHE["nc"] = nc
    return nc


def kernel(x, Wq, bq, Wk, bk, Wv, bv, Wo, bo):
    import ml_dtypes
    x = np.asarray(x, dtype=np.float32)
    nc = _build_nc()

    wqT = np.ascontiguousarray(np.asarray(Wq, np.float32).T)
    wkT = np.ascontiguousarray(np.asarray(Wk, np.float32).T)
    wvT = np.ascontiguousarray(np.asarray(Wv, np.float32).T)
    woT = np.ascontiguousarray(
        np.asarray(Wo, np.float32).T).astype(ml_dtypes.bfloat16)
    bq2 = np.asarray(bq, np.float32).reshape(1, D)
    bk2 = np.asarray(bk, np.float32).reshape(1, D)
    bv2 = np.asarray(bv, np.float32).reshape(1, D)
    bo2 = np.asarray(bo, np.float32).reshape(1, D).astype(ml_dtypes.bfloat16)

    in_maps = []
    for core in range(NCORES):
        b, qh = core // 2, core % 2
        xTb = np.ascontiguousarray(x[b].T)
        in_maps.append({
            "xT": xTb,
            "xqT": np.ascontiguousarray(xTb[:, qh * NQ:(qh + 1) * NQ]),
            "wqT": wqT, "wkT": wkT, "wvT": wvT, "woT": woT,
            "bq": bq2, "bk": bk2, "bv": bv2, "bo": bo2,
        })

    res = bass_utils.run_bass_kernel_spmd(nc, in_maps, list(range(NCORES)))
    out = np.empty((B, N, D), np.float32)
    for core in range(NCORES):
        b, qh = core // 2, core % 2
        out[b, qh * NQ:(qh + 1) * NQ, :] = res.results[core]["out"]
    return out
